# revision 1
# baseline (speedup 1.0000x reference)
"""ArcFace loss kernel for 8 Trainium2 NeuronCores.

Model-parallel over the identities axis (I=100000 -> 12500 per core):
  pass 1: local sum(w^2) over identities -> split AllReduce (overlapped)
          -> inv norms folded into bf16 embeddings
  pass 2: logits = 64*cos(theta + margin*onehot) via bf16 matmuls,
          row sums of exp(logit - 20) via ACT accumulators,
          logits stashed to DRAM as bf16
  split AllReduce row sums -> logsumexp
  pass 3: out = logits - logsumexp
"""

import math
import sys

if "/opt/trn_rl_repo" not in sys.path:
    sys.path.insert(0, "/opt/trn_rl_repo")

import numpy as np
import ml_dtypes

import concourse.mybir as mybir
from concourse import bacc, tile
from concourse.alu_op_type import AluOpType
from concourse.bass_utils import run_bass_kernel_spmd

NCORES = 8
B, E, I, S = 512, 512, 100000, 3
IL = I // NCORES      # identities per core
IT = 500              # identities per matmul tile
NIT = IL // IT        # 25 matmul i-tiles
W2T = 2500            # identities per w DMA tile (flat, per (s, e-chunk))
NW2 = IL // W2T       # 5
JT = W2T // IT        # 5 matmul tiles per w tile
BC = B // 128         # batch chunks of 128
EC = E // 128         # embedding chunks of 128

MARGIN = 0.5
SCALE = 64.0
C0 = 20.0                           # fixed exp shift (|logit| <= ~25 for this data)
K1_64 = 1.0 - math.cos(MARGIN)      # (SCALE*(1-cos m))/SCALE
K2 = SCALE * math.sin(MARGIN)
EPS = 1e-12

F32 = mybir.dt.float32
BF16 = mybir.dt.bfloat16
X = mybir.AxisListType.X

_cache = {}


def _build():
    nc = bacc.Bacc("TRN2", target_bir_lowering=False, debug=False,
                   num_devices=NCORES)
    wt = nc.dram_tensor("wt", [S * E, IL], F32, kind="ExternalInput").ap()
    embT = nc.dram_tensor("embT", [E, B], F32, kind="ExternalInput").ap()
    tgt = nc.dram_tensor("tgt", [B, IL], BF16, kind="ExternalInput").ap()
    out = nc.dram_tensor("out", [B, IL], F32, kind="ExternalOutput").ap()

    rg = [list(range(NCORES))]

    with tile.TileContext(nc) as tc:
        from contextlib import ExitStack
        with ExitStack() as st:
            p_const = st.enter_context(tc.tile_pool(name="const", bufs=1))
            p_w = st.enter_context(tc.tile_pool(name="w", bufs=9))
            p_t = st.enter_context(tc.tile_pool(name="tp", bufs=2))
            p_m64 = st.enter_context(tc.tile_pool(name="m64", bufs=3))
            p_work = st.enter_context(tc.tile_pool(name="work", bufs=3))
            p_hm = st.enter_context(tc.tile_pool(name="hm", bufs=2))
            p_p3i = st.enter_context(tc.tile_pool(name="p3i", bufs=7))
            p_p3o = st.enter_context(tc.tile_pool(name="p3o", bufs=2))
            p_psum = st.enter_context(tc.tile_pool(name="ps", bufs=8, space="PSUM"))
            p_dram = st.enter_context(tc.tile_pool(name="dram", bufs=1, space="DRAM"))

            # bias constants for activations (float bias needs a const AP)
            bias_k22 = p_const.tile([128, 1], F32)
            nc.vector.memset(bias_k22[:], K2 * K2)
            bias_nc0 = p_const.tile([128, 1], F32)
            nc.vector.memset(bias_nc0[:], -C0)

            # ---------------- pass 1: sum of squares over local identities,
            # split in two chunks so the first AllReduce overlaps the rest
            P1T = 6250
            NP1 = IL // P1T                # 2 i-chunks
            CHA = 1                        # chunk A = i-chunk 0
            s2parts = p_const.tile([128, S * EC * NP1], F32)
            ar1_in = [p_dram.tile([128, S * EC], F32, name=f"ar1i{h}")
                      for h in range(2)]
            ar1_out = [p_dram.tile([128, S * EC], F32, name=f"ar1o{h}")
                       for h in range(2)]
            sumsq = [p_const.tile([128, S * EC], F32, name=f"sumsq{h}")
                     for h in range(2)]
            for half, itgs in ((0, range(CHA)), (1, range(CHA, NP1))):
                for itg in itgs:
                    i0 = itg * P1T
                    for s in range(S):
                        for c in range(EC):
                            w1 = p_w.tile([128, P1T], BF16, name="wtile")
                            nc.gpsimd.dma_start(
                                w1[:],
                                wt[s * E + c * 128:s * E + (c + 1) * 128,
                                   i0:i0 + P1T])
                            col = (s * EC + c) * NP1 + itg
                            nc.scalar.activation(
                                w1[:], w1[:],
                                mybir.ActivationFunctionType.Square,
                                accum_out=s2parts[:, col:col + 1])
                lo = itgs[0]
                n = len(itgs)
                for j in range(S * EC):
                    nc.vector.tensor_reduce(
                        sumsq[half][:, j:j + 1],
                        s2parts[:, j * NP1 + lo:j * NP1 + lo + n],
                        X, AluOpType.add)
                nc.sync.dma_start(ar1_in[half][:], sumsq[half][:])
                nc.gpsimd.collective_compute(
                    "AllReduce", AluOpType.add, replica_groups=rg,
                    ins=[ar1_in[half].opt()], outs=[ar1_out[half].opt()])

            gssp = p_const.tile([128, S * EC, 2], F32)
            for h in range(2):
                nc.sync.dma_start(gssp[:, :, h], ar1_out[h][:])
            gss = p_const.tile([128, S * EC], F32)
            nc.vector.tensor_reduce(gss[:], gssp[:], X, AluOpType.add)

            norm = p_const.tile([128, S * EC], F32)
            nc.scalar.activation(norm[:], gss[:],
                                 mybir.ActivationFunctionType.Sqrt)
            nc.vector.tensor_scalar_max(norm[:], norm[:], EPS)
            inv = p_const.tile([128, S * EC], F32)
            nc.vector.reciprocal(inv[:], norm[:])
            # one newton step: inv = inv*(2 - norm*inv)
            nt = p_const.tile([128, S * EC], F32)
            nc.vector.scalar_tensor_tensor(nt[:], norm[:], 0.0, inv[:],
                                           AluOpType.bypass, AluOpType.mult)
            nc.vector.tensor_scalar(nt[:], nt[:], -1.0, 2.0,
                                    AluOpType.mult, AluOpType.add)
            nc.vector.scalar_tensor_tensor(inv[:], inv[:], 0.0, nt[:],
                                           AluOpType.bypass, AluOpType.mult)

            # ---------------- scaled transposed embeddings, bf16
            embT_sb = p_const.tile([128, EC, B], F32)
            nc.sync.dma_start(embT_sb[:], embT.rearrange("(c p) b -> p c b", p=128))
            embS = []
            for s in range(S):
                es = p_const.tile([128, EC, B], BF16, name=f"embS{s}")
                for c in range(EC):
                    nc.vector.tensor_scalar(
                        es[:, c, :], embT_sb[:, c, :],
                        inv[:, s * EC + c:s * EC + c + 1], SCALE,
                        AluOpType.mult, AluOpType.mult)
                embS.append(es)

            # ---------------- pass 2: matmuls, margin, exp-sums, stash
            SC = W2T                    # stash chunk width (2500)
            stash = [p_dram.tile([B, SC], BF16, name=f"stash{h}")
                     for h in range(NW2)]
            sexp_parts = p_const.tile([128, BC * NIT], F32)
            for it in range(NIT):
                    i0 = it * IT
                    wsit = p_w.tile([128, S, EC, IT], BF16, name="wtile")
                    nc.gpsimd.dma_start(
                        wsit[:],
                        wt[:, i0:i0 + IT]
                        .rearrange("(s c p) i -> p s c i", s=S, p=128))
                    m64 = p_m64.tile([128, BC, IT], BF16, name="m64")
                    work = p_work.tile([128, BC, IT], BF16, name="work")
                    ttile = p_t.tile([128, BC, IT], BF16, name="ttile")
                    nc.sync.dma_start(
                        ttile[:],
                        tgt[:, i0:i0 + IT].rearrange("(b p) i -> p b i", p=128))
                    for b in range(BC):
                        pss = []
                        for s in range(S):
                            ps = p_psum.tile([128, IT], F32, name="ps")
                            for c in range(EC):
                                nc.tensor.matmul(
                                    ps[:],
                                    embS[s][:, c, b * 128:(b + 1) * 128],
                                    wsit[:, s, c, :],
                                    start=(c == 0), stop=(c == EC - 1))
                            pss.append(ps)
                        dst = m64[:, b, :]
                        nc.vector.tensor_copy(dst, pss[0][:])
                        nc.vector.tensor_max(dst, pss[1][:], dst)
                        nc.vector.tensor_max(dst, pss[2][:], dst)
                    # m64 = 64*cos. work = sqrt(K2^2 - (K2/64)^2 m64^2) = K2 sin
                    nc.scalar.activation(work[:], m64[:],
                                         mybir.ActivationFunctionType.Square,
                                         scale=1.0 / SCALE)
                    nc.scalar.activation(work[:], work[:],
                                         mybir.ActivationFunctionType.Sqrt,
                                         bias=bias_k22[:], scale=-(K2 * K2))
                    # work = K1/64 * m64 + K2*sin(theta)   (ts 4x + tt 2x, bf16)
                    hm = p_hm.tile([128, BC, IT], BF16, name="hm")
                    nc.vector.tensor_scalar_mul(hm[:], m64[:], K1_64)
                    nc.vector.tensor_add(work[:], hm[:], work[:])
                    # work = work * target ; logits (into m64) = m64 - work
                    nc.vector.tensor_mul(work[:], work[:], ttile[:])
                    nc.vector.tensor_sub(m64[:], m64[:], work[:])
                    # per-b exp(logits - C0), accumulate row sums
                    for b in range(BC):
                        nc.scalar.activation(
                            work[:, b, :], m64[:, b, :],
                            mybir.ActivationFunctionType.Exp, bias=bias_nc0[:],
                            accum_out=sexp_parts[:, b * NIT + it:b * NIT + it + 1])
                    sdst = stash[i0 // SC][:, i0 % SC:i0 % SC + IT]
                    nc.scalar.dma_start(
                        sdst.rearrange("(b p) i -> p b i", p=128), m64[:])

            # ---------------- split allreduce of row sums -> logsumexp
            ITS_S = 20                  # first sexp chunk: i-tiles 0..19
            ar2_in = [p_dram.tile([128, BC], F32, name=f"ar2i{h}")
                      for h in range(2)]
            ar2_out = [p_dram.tile([128, BC], F32, name=f"ar2o{h}")
                       for h in range(2)]
            slocA = p_const.tile([128, BC], F32)
            for b in range(BC):
                nc.vector.tensor_reduce(
                    slocA[:, b:b + 1],
                    sexp_parts[:, b * NIT:b * NIT + ITS_S],
                    X, AluOpType.add)
            nc.sync.dma_start(ar2_in[0][:], slocA[:])
            nc.gpsimd.collective_compute(
                "AllReduce", AluOpType.add, replica_groups=rg,
                ins=[ar2_in[0].opt()], outs=[ar2_out[0].opt()])
            slocB = p_const.tile([128, BC], F32)
            for b in range(BC):
                nc.vector.tensor_reduce(
                    slocB[:, b:b + 1],
                    sexp_parts[:, b * NIT + ITS_S:(b + 1) * NIT],
                    X, AluOpType.add)
            nc.sync.dma_start(ar2_in[1][:], slocB[:])
            nc.gpsimd.collective_compute(
                "AllReduce", AluOpType.add, replica_groups=rg,
                ins=[ar2_in[1].opt()], outs=[ar2_out[1].opt()])
            sgp = p_const.tile([128, BC, 2], F32)
            for h in range(2):
                nc.sync.dma_start(sgp[:, :, h], ar2_out[h][:])
            sg = p_const.tile([128, BC], F32)
            nc.vector.tensor_reduce(sg[:], sgp[:], X, AluOpType.add)
            lse = p_const.tile([128, BC], F32)
            nc.scalar.activation(lse[:], sg[:], mybir.ActivationFunctionType.Ln)

            # ---------------- pass 3: out = logits - lse - C0
            J = 1250
            for h in range(NW2):
                base = h * SC
                for b in range(BC):
                    for j in range(SC // J):
                        lt = p_p3i.tile([128, J], BF16, name="lt")
                        nc.sync.dma_start(
                            lt[:], stash[h][b * 128:(b + 1) * 128,
                                           j * J:(j + 1) * J])
                        lo = p_p3o.tile([128, J], F32, name="lo")
                        nc.vector.tensor_scalar(
                            lo[:], lt[:], lse[:, b:b + 1], C0,
                            AluOpType.subtract, AluOpType.subtract)
                        nc.scalar.dma_start(
                            out[b * 128:(b + 1) * 128,
                                base + j * J:base + (j + 1) * J], lo[:])

    nc.compile()
    return nc


def _get_nc():
    if "nc" not in _cache:
        _cache["nc"] = _build()
    return _cache["nc"]


def _shard(embedding_batch, target_batch, w):
    embT = np.ascontiguousarray(embedding_batch.T, dtype=np.float32)
    # (E, I, S) -> (S, E, I) once, then contiguous per-core slices
    wT = np.ascontiguousarray(np.transpose(w, (2, 0, 1)), dtype=np.float32)
    in_maps = []
    for k in range(NCORES):
        lo, hi = k * IL, (k + 1) * IL
        in_maps.append({
            "wt": np.ascontiguousarray(wT[:, :, lo:hi]).reshape(S * E, IL),
            "embT": embT,
            "tgt": np.ascontiguousarray(target_batch[:, lo:hi]).astype(ml_dtypes.bfloat16),
        })
    return in_maps


def run_sharded(embedding_batch, target_batch, w, trace=False, trace_kwargs=None):
    nc = _get_nc()
    in_maps = _shard(embedding_batch, target_batch, w)
    res = run_bass_kernel_spmd(nc, in_maps, core_ids=list(range(NCORES)),
                               trace=trace, **(trace_kwargs or {}))
    full = np.concatenate([res.results[k]["out"] for k in range(NCORES)], axis=1)
    return full, res


def kernel(embedding_batch, target_batch, w):
    full, _ = run_sharded(embedding_batch, target_batch, w)
    return full

